# revision 20
# baseline (speedup 1.0000x reference)
"""Trainium2 Bass kernel for nn_ClusteringLayer (vq_codebook, Student-t assignments).

Computes, for x in R^{N x D} and clusters c in R^{K x D}:
    d2[n,k] = ||x_n - c_k||^2
    q = 1 / (1 + d2)            (Student-t, alpha=1, power=(alpha+1)/2=1)
    out = q / q.sum(-1, keepdims=True)

Strategy (data-parallel over 8 NeuronCores, cluster table replicated):
  - host: shard x along N (65536 rows/core); each shard ships as an augmented
    transposed tile X~ [260, Nsh] bf16 whose rows are
        [ x^T (256) ; x2 - 256 ; 1 ; 1 ; 0 ]
    so the whole Student-t numerator 1 + d2 comes out of ONE matmul chain:
        psum[n,k] = sum_r X~[r,n] * M[r,k]
    with moving M = [ -2 c^T (256) ; 1 ; A ; B ; 0 ] where A + B is a
    two-term bf16 split of (257 + ||c_k||^2)  (so the bf16 rounding error of
    the large constant cancels to ~1e-2 absolute).
  - device, per 128-row n-tile, 3 matmuls with the X~ slice STATIONARY
    (psum lands in [n, k] layout directly -> no transpose anywhere):
        mm1: stat xt[0:128,t]   x mov ct0 [128,64]
        mm2: stat xt[128:256,t] x mov ct1 [128,64]
        mm3: stat xt[256:260,t] x mov ct2 [4,64]
    8 tiles accumulate into one PSUM bank [128, 8*64] as a single
    accumulation group (start on the first MM, stop on the last).
  - ACT: q = Reciprocal(psum) -> bf16 (one table set, loaded once)
  - GPSIMD: s = sum_k q ; DVE: sinv = 1/s, out = q * sinv -> bf16
  - out is stored in the SBUF-natural [p, s, k] layout (contiguous 4KB DMA
    lines); host inverts the (s p) interleave when unsharding.
"""

import numpy as np
from contextlib import ExitStack


def _patch_act_tables():
    """Make Ln and Exp resolve to the single set that contains both
    (natural_log_exp_and_others), so the kernel pays one ACT_TABLE_LOAD
    instead of alternating sets per instruction.  Only values are modified --
    set order (and hence act_func_set_id indices) is preserved."""
    import functools
    from concourse import hw_specs, bacc, mybir

    if getattr(hw_specs, "_act_tables_patched", False):
        return
    orig = hw_specs.get_activation_tables

    @functools.cache
    def patched(arch):
        t = dict(orig(arch))
        ln = mybir.ActivationFunctionType.Ln
        ex = mybir.ActivationFunctionType.Exp
        out = {}
        for name, funcs in t.items():
            if name != "natural_log_exp_and_others" and (ln in funcs or ex in funcs):
                funcs = funcs - {ln, ex}
            out[name] = funcs
        return out

    hw_specs.get_activation_tables = patched
    bacc.get_activation_tables = patched
    hw_specs._act_tables_patched = True

N, D, K = 524288, 256, 64
NCORES = 8
NSH = N // NCORES          # 65536 rows per core
XROWS = D + 4              # 256 data rows + [x2-256, 1, 1, 0]
BLK = 4096                 # n-columns per DMA block
CHUNK = 1024               # n-columns per PSUM bank (8 tiles of 128)
NT = NSH // 128            # 512 n-tiles per core


def _build(nsh=NSH, blk=BLK):
    import concourse.bacc as bacc
    import concourse.tile as tile
    from concourse import mybir

    _patch_act_tables()

    f32 = mybir.dt.float32
    bf16 = mybir.dt.bfloat16
    f16 = mybir.dt.float16
    nblk = nsh // blk
    tb = blk // 128            # n-tiles per block (32)
    nch = blk // CHUNK         # psum banks per block (4)
    ct = CHUNK // 128          # n-tiles per bank (8)

    nc = bacc.Bacc("TRN2", target_bir_lowering=False, debug=False)
    xt = nc.dram_tensor("xt", [XROWS, nsh], bf16, kind="ExternalInput").ap()
    cl = nc.dram_tensor("clusters", [K, D], f32, kind="ExternalInput").ap()
    # output in SBUF-natural layout: q_dev[p, s*K + k] = q[s*128 + p, k]
    qo = nc.dram_tensor("q", [128, (nsh // 128) * K], bf16,
                        kind="ExternalOutput").ap()

    with tile.TileContext(nc) as tc, ExitStack() as ctx:
        wp = ctx.enter_context(tc.tile_pool(name="w", bufs=1))
        io = ctx.enter_context(tc.tile_pool(name="io", bufs=4))
        qp = ctx.enter_context(tc.tile_pool(name="qp", bufs=2))
        up = ctx.enter_context(tc.tile_pool(name="up", bufs=3))
        sp = ctx.enter_context(tc.tile_pool(name="sp", bufs=3))
        op = ctx.enter_context(tc.tile_pool(name="op", bufs=2))
        # one psum tile = 1 bank = 8 n-tiles; 6 bufs rotate so PE can run
        # several banks ahead of the ACT drain (fine-grained pipelining
        # measured faster than 4-bank tiles).
        pp = ctx.enter_context(tc.tile_pool(name="ps", bufs=6, space="PSUM"))

        # --- one-time cluster prep (replicated on every core) ---
        cl_sb = wp.tile([K, D], f32, tag="cl")
        nc.sync.dma_start(cl_sb, cl)
        csq = wp.tile([K, D], f32, tag="csq")
        nc.vector.tensor_mul(csq, cl_sb, cl_sb)
        c2 = wp.tile([K, 1], f32, tag="c2")
        nc.vector.tensor_reduce(c2, csq, axis=mybir.AxisListType.X,
                                op=mybir.AluOpType.add)
        t257 = wp.tile([K, 1], f32, tag="t257")
        nc.vector.tensor_scalar_add(t257, c2, 257.0)
        ab = wp.tile([K, 1], bf16, tag="ab")
        nc.vector.tensor_copy(ab, t257)
        a32 = wp.tile([K, 1], f32, tag="a32")
        nc.vector.tensor_copy(a32, ab)
        bres = wp.tile([K, 1], f32, tag="bres")
        nc.vector.tensor_sub(bres, t257, a32)
        bb = wp.tile([K, 1], bf16, tag="bb")
        nc.vector.tensor_copy(bb, bres)
        # colsrc columns [1, A, B, 0...] -> transpose -> ct2 rows
        colsrc = wp.tile([K, 128], bf16, tag="colsrc")
        nc.vector.memset(colsrc, 0.0)
        nc.vector.memset(colsrc[:, 0:1], 1.0)
        nc.vector.tensor_copy(colsrc[:, 1:2], ab)
        nc.vector.tensor_copy(colsrc[:, 2:3], bb)
        ct2 = wp.tile([128, K], bf16, tag="ct2")
        nc.sync.dma_start_transpose(ct2, colsrc)
        # ct0/ct1 = (-2 c)^T halves
        cn2 = wp.tile([K, D], bf16, tag="cn2")
        nc.vector.tensor_scalar_mul(cn2, cl_sb, -2.0)
        ct0 = wp.tile([128, K], bf16, tag="ct0")
        ct1 = wp.tile([128, K], bf16, tag="ct1")
        nc.sync.dma_start_transpose(ct0, cn2[:, 0:128])
        nc.sync.dma_start_transpose(ct1, cn2[:, 128:256])

        for b in range(nblk):
            n0 = b * blk
            xt0 = io.tile([128, blk], bf16, tag="xt0")
            xt1 = io.tile([128, blk], bf16, tag="xt1")
            xt2 = io.tile([4, blk], bf16, tag="xt2")
            # split the two 1MB loads across both HWDGE rings (SP + ACT)
            nc.sync.dma_start(xt0, xt[0:128, n0:n0 + blk])
            nc.scalar.dma_start(xt1, xt[128:256, n0:n0 + blk])
            nc.sync.dma_start(xt2, xt[256:260, n0:n0 + blk])

            qblk = qp.tile([128, tb, K], bf16, tag="qb")
            s = sp.tile([128, tb, 1], f32, tag="s")
            sinv = sp.tile([128, tb, 1], f32, tag="sinv")
            sinvb = sp.tile([128, tb, 1], bf16, tag="sinvb")
            outb = op.tile([128, tb, K], bf16, tag="outb")

            for c in range(nch):
                ps = pp.tile([128, ct * K], f32, tag="d2")
                for t in range(ct):
                    off = (c * ct + t) * 128
                    sl = slice(t * K, (t + 1) * K)
                    nc.tensor.matmul(ps[:, sl], xt0[:, off:off + 128], ct0,
                                     start=(t == 0), stop=False)
                    nc.tensor.matmul(ps[:, sl], xt1[:, off:off + 128], ct1,
                                     start=False, stop=False)
                    nc.tensor.matmul(ps[:, sl], xt2[:, off:off + 128],
                                     ct2[0:4, :], start=False,
                                     stop=(t == ct - 1))

                tsl = slice(c * ct, (c + 1) * ct)
                psv = ps.rearrange("p (t k) -> p t k", k=K)
                # q = exp(-ln(1+d2)) = 1/(1+d2); ACT Reciprocal is blocked
                # (HW accuracy), Ln+Exp share one table set.
                u = up.tile([128, ct, K], f16, tag="u")
                nc.scalar.activation(u, psv,
                                     func=mybir.ActivationFunctionType.Ln,
                                     scale=1.0)
                nc.scalar.activation(qblk[:, tsl, :], u,
                                     func=mybir.ActivationFunctionType.Exp,
                                     scale=-1.0)
                nc.vector.tensor_reduce(s[:, tsl, :], qblk[:, tsl, :],
                                        axis=mybir.AxisListType.X,
                                        op=mybir.AluOpType.add)
                nc.vector.reciprocal(sinv[:, tsl, :], s[:, tsl, :])
                nc.vector.tensor_copy(sinvb[:, tsl, :], sinv[:, tsl, :])
                nc.vector.tensor_tensor(
                    outb[:, tsl, :], qblk[:, tsl, :],
                    sinvb[:, tsl, :].to_broadcast([128, ct, K]),
                    op=mybir.AluOpType.mult)

            ov = outb.rearrange("p t k -> p (t k)")
            nc.scalar.dma_start(qo[:, b * tb * K:(b + 1) * tb * K], ov)

    nc.compile()
    return nc


_CACHE = {}


def _get_nc(nsh=NSH, blk=BLK):
    key = (nsh, blk)
    if key not in _CACHE:
        _CACHE[key] = _build(nsh, blk)
    return _CACHE[key]


def prep_in_maps(inputs, clusters):
    """Host-side shard/layout prep shared by kernel() and test harnesses."""
    import ml_dtypes

    x = np.asarray(inputs, dtype=np.float32)
    c = np.ascontiguousarray(np.asarray(clusters, dtype=np.float32))
    assert x.shape == (N, D) and c.shape == (K, D)

    in_maps = []
    for i in range(NCORES):
        xs = x[i * NSH:(i + 1) * NSH]
        aug = np.empty((XROWS, NSH), dtype=ml_dtypes.bfloat16)
        aug[0:D] = xs.T
        aug[D] = (xs * xs).sum(axis=1) - 256.0
        aug[D + 1] = 1.0
        aug[D + 2] = 1.0
        aug[D + 3] = 0.0
        in_maps.append({"xt": np.ascontiguousarray(aug), "clusters": c})
    return in_maps


def unshard(results):
    """[128, (NSH//128)*K] bf16 per core -> [N, K] f32."""
    outs = []
    for r in results:
        qd = np.asarray(r["q"]).reshape(128, NSH // 128, K)
        outs.append(qd.transpose(1, 0, 2).reshape(NSH, K).astype(np.float32))
    return np.concatenate(outs, axis=0)


def kernel(inputs, clusters):
    from concourse.bass_utils import run_bass_kernel_spmd

    nc = _get_nc()
    in_maps = prep_in_maps(inputs, clusters)
    res = run_bass_kernel_spmd(nc, in_maps, core_ids=list(range(NCORES)))
    return unshard(res.results)


# revision 21
# speedup vs baseline: 1.1865x; 1.1865x over previous
"""Trainium2 Bass kernel for nn_ClusteringLayer (vq_codebook, Student-t assignments).

Computes, for x in R^{N x D} and clusters c in R^{K x D}:
    d2[n,k] = ||x_n - c_k||^2
    q = 1 / (1 + d2)            (Student-t, alpha=1, power=(alpha+1)/2=1)
    out = q / q.sum(-1, keepdims=True)

Strategy (data-parallel over 8 NeuronCores, cluster table replicated):
  - host: shard x along N (65536 rows/core); each shard ships as an augmented
    transposed tile X~ [260, Nsh] bf16 whose rows are
        [ x^T (256) ; x2 - 256 ; 1 ; 1 ; 0 ]
    so the whole Student-t numerator 1 + d2 comes out of ONE matmul chain:
        psum[n,k] = sum_r X~[r,n] * M[r,k]
    with moving M = [ -2 c^T (256) ; 1 ; A ; B ; 0 ] where A + B is a
    two-term bf16 split of (257 + ||c_k||^2)  (so the bf16 rounding error of
    the large constant cancels to ~1e-2 absolute).
  - device, per 128-row n-tile, 3 matmuls with the X~ slice STATIONARY
    (psum lands in [n, k] layout directly -> no transpose anywhere):
        mm1: stat xt[0:128,t]   x mov ct0 [128,64]
        mm2: stat xt[128:256,t] x mov ct1 [128,64]
        mm3: stat xt[256:260,t] x mov ct2 [4,64]
    8 tiles accumulate into one PSUM bank [128, 8*64] as a single
    accumulation group (start on the first MM, stop on the last).
  - ACT: q = Reciprocal(psum) -> bf16 (one table set, loaded once)
  - GPSIMD: s = sum_k q ; DVE: sinv = 1/s, out = q * sinv -> bf16
  - out is stored in the SBUF-natural [p, s, k] layout (contiguous 4KB DMA
    lines); host inverts the (s p) interleave when unsharding.
"""

import numpy as np
from contextlib import ExitStack


def _patch_act_tables():
    """Make Ln and Exp resolve to the single set that contains both
    (natural_log_exp_and_others), so the kernel pays one ACT_TABLE_LOAD
    instead of alternating sets per instruction.  Only values are modified --
    set order (and hence act_func_set_id indices) is preserved."""
    import functools
    from concourse import hw_specs, bacc, mybir

    if getattr(hw_specs, "_act_tables_patched", False):
        return
    orig = hw_specs.get_activation_tables

    @functools.cache
    def patched(arch):
        t = dict(orig(arch))
        ln = mybir.ActivationFunctionType.Ln
        ex = mybir.ActivationFunctionType.Exp
        out = {}
        for name, funcs in t.items():
            if name != "natural_log_exp_and_others" and (ln in funcs or ex in funcs):
                funcs = funcs - {ln, ex}
            out[name] = funcs
        return out

    hw_specs.get_activation_tables = patched
    bacc.get_activation_tables = patched
    hw_specs._act_tables_patched = True

N, D, K = 524288, 256, 64
NCORES = 8
NSH = N // NCORES          # 65536 rows per core
XROWS = D + 4              # 256 data rows + [x2-256, 1, 1, 0]
BLK = 4096                 # n-columns per DMA block
CHUNK = 1024               # n-columns per PSUM bank (8 tiles of 128)
NT = NSH // 128            # 512 n-tiles per core


def _build(nsh=NSH, blk=BLK):
    import concourse.bacc as bacc
    import concourse.tile as tile
    from concourse import mybir

    _patch_act_tables()

    f32 = mybir.dt.float32
    bf16 = mybir.dt.bfloat16
    f16 = mybir.dt.float16
    nblk = nsh // blk
    tb = blk // 128            # n-tiles per block (32)
    nch = blk // CHUNK         # psum banks per block (4)
    ct = CHUNK // 128          # n-tiles per bank (8)

    nc = bacc.Bacc("TRN2", target_bir_lowering=False, debug=False)
    xt = nc.dram_tensor("xt", [XROWS, nsh], bf16, kind="ExternalInput").ap()
    cl = nc.dram_tensor("clusters", [K, D], f32, kind="ExternalInput").ap()
    # output in SBUF-natural layout: q_dev[p, s*K + k] = q[s*128 + p, k]
    qo = nc.dram_tensor("q", [128, (nsh // 128) * K], bf16,
                        kind="ExternalOutput").ap()

    with tile.TileContext(nc) as tc, ExitStack() as ctx:
        wp = ctx.enter_context(tc.tile_pool(name="w", bufs=1))
        io = ctx.enter_context(tc.tile_pool(name="io", bufs=3))
        qp = ctx.enter_context(tc.tile_pool(name="qp", bufs=2))
        up = ctx.enter_context(tc.tile_pool(name="up", bufs=3))
        sp = ctx.enter_context(tc.tile_pool(name="sp", bufs=3))
        op = ctx.enter_context(tc.tile_pool(name="op", bufs=2))
        # one psum tile = 1 bank = 8 n-tiles; 4 bufs rotate so PE fills bank
        # b+1..b+3 while ACT drains bank b (fine-grained pipelining measured
        # faster than 4-bank tiles).
        pp = ctx.enter_context(tc.tile_pool(name="ps", bufs=4, space="PSUM"))

        # --- one-time cluster prep (replicated on every core) ---
        cl_sb = wp.tile([K, D], f32, tag="cl")
        nc.sync.dma_start(cl_sb, cl)
        csq = wp.tile([K, D], f32, tag="csq")
        nc.vector.tensor_mul(csq, cl_sb, cl_sb)
        c2 = wp.tile([K, 1], f32, tag="c2")
        nc.vector.tensor_reduce(c2, csq, axis=mybir.AxisListType.X,
                                op=mybir.AluOpType.add)
        t257 = wp.tile([K, 1], f32, tag="t257")
        nc.vector.tensor_scalar_add(t257, c2, 257.0)
        ab = wp.tile([K, 1], bf16, tag="ab")
        nc.vector.tensor_copy(ab, t257)
        a32 = wp.tile([K, 1], f32, tag="a32")
        nc.vector.tensor_copy(a32, ab)
        bres = wp.tile([K, 1], f32, tag="bres")
        nc.vector.tensor_sub(bres, t257, a32)
        bb = wp.tile([K, 1], bf16, tag="bb")
        nc.vector.tensor_copy(bb, bres)
        # colsrc columns [1, A, B, 0...] -> transpose -> ct2 rows
        colsrc = wp.tile([K, 128], bf16, tag="colsrc")
        nc.vector.memset(colsrc, 0.0)
        nc.vector.memset(colsrc[:, 0:1], 1.0)
        nc.vector.tensor_copy(colsrc[:, 1:2], ab)
        nc.vector.tensor_copy(colsrc[:, 2:3], bb)
        ct2 = wp.tile([128, K], bf16, tag="ct2")
        nc.sync.dma_start_transpose(ct2, colsrc)
        # ct0/ct1 = (-2 c)^T halves
        cn2 = wp.tile([K, D], bf16, tag="cn2")
        nc.vector.tensor_scalar_mul(cn2, cl_sb, -2.0)
        ct0 = wp.tile([128, K], bf16, tag="ct0")
        ct1 = wp.tile([128, K], bf16, tag="ct1")
        nc.sync.dma_start_transpose(ct0, cn2[:, 0:128])
        nc.sync.dma_start_transpose(ct1, cn2[:, 128:256])

        for b in range(nblk):
            n0 = b * blk
            xt0 = io.tile([128, blk], bf16, tag="xt0")
            xt1 = io.tile([128, blk], bf16, tag="xt1")
            xt2 = io.tile([4, blk], bf16, tag="xt2")
            nc.sync.dma_start(xt0, xt[0:128, n0:n0 + blk])
            nc.sync.dma_start(xt1, xt[128:256, n0:n0 + blk])
            nc.sync.dma_start(xt2, xt[256:260, n0:n0 + blk])

            qblk = qp.tile([128, tb, K], bf16, tag="qb")
            s = sp.tile([128, tb, 1], f32, tag="s")
            sinv = sp.tile([128, tb, 1], f32, tag="sinv")
            sinvb = sp.tile([128, tb, 1], bf16, tag="sinvb")
            outb = op.tile([128, tb, K], bf16, tag="outb")

            for c in range(nch):
                ps = pp.tile([128, ct * K], f32, tag="d2")
                for t in range(ct):
                    off = (c * ct + t) * 128
                    sl = slice(t * K, (t + 1) * K)
                    nc.tensor.matmul(ps[:, sl], xt0[:, off:off + 128], ct0,
                                     start=(t == 0), stop=False)
                    nc.tensor.matmul(ps[:, sl], xt1[:, off:off + 128], ct1,
                                     start=False, stop=False)
                    nc.tensor.matmul(ps[:, sl], xt2[:, off:off + 128],
                                     ct2[0:4, :], start=False,
                                     stop=(t == ct - 1))

                tsl = slice(c * ct, (c + 1) * ct)
                psv = ps.rearrange("p (t k) -> p t k", k=K)
                # q = exp(-ln(1+d2)) = 1/(1+d2); ACT Reciprocal is blocked
                # (HW accuracy), Ln+Exp share one table set.
                u = up.tile([128, ct, K], f16, tag="u")
                nc.scalar.activation(u, psv,
                                     func=mybir.ActivationFunctionType.Ln,
                                     scale=1.0)
                nc.scalar.activation(qblk[:, tsl, :], u,
                                     func=mybir.ActivationFunctionType.Exp,
                                     scale=-1.0)
                nc.vector.tensor_reduce(s[:, tsl, :], qblk[:, tsl, :],
                                        axis=mybir.AxisListType.X,
                                        op=mybir.AluOpType.add)
                nc.vector.reciprocal(sinv[:, tsl, :], s[:, tsl, :])
                nc.vector.tensor_copy(sinvb[:, tsl, :], sinv[:, tsl, :])
                nc.vector.tensor_tensor(
                    outb[:, tsl, :], qblk[:, tsl, :],
                    sinvb[:, tsl, :].to_broadcast([128, ct, K]),
                    op=mybir.AluOpType.mult)

            ov = outb.rearrange("p t k -> p (t k)")
            nc.scalar.dma_start(qo[:, b * tb * K:(b + 1) * tb * K], ov)

    nc.compile()
    return nc


_CACHE = {}


def _get_nc(nsh=NSH, blk=BLK):
    key = (nsh, blk)
    if key not in _CACHE:
        _CACHE[key] = _build(nsh, blk)
    return _CACHE[key]


def prep_in_maps(inputs, clusters):
    """Host-side shard/layout prep shared by kernel() and test harnesses."""
    import ml_dtypes

    x = np.asarray(inputs, dtype=np.float32)
    c = np.ascontiguousarray(np.asarray(clusters, dtype=np.float32))
    assert x.shape == (N, D) and c.shape == (K, D)

    in_maps = []
    for i in range(NCORES):
        xs = x[i * NSH:(i + 1) * NSH]
        aug = np.empty((XROWS, NSH), dtype=ml_dtypes.bfloat16)
        aug[0:D] = xs.T
        aug[D] = (xs * xs).sum(axis=1) - 256.0
        aug[D + 1] = 1.0
        aug[D + 2] = 1.0
        aug[D + 3] = 0.0
        in_maps.append({"xt": np.ascontiguousarray(aug), "clusters": c})
    return in_maps


def unshard(results):
    """[128, (NSH//128)*K] bf16 per core -> [N, K] f32."""
    outs = []
    for r in results:
        qd = np.asarray(r["q"]).reshape(128, NSH // 128, K)
        outs.append(qd.transpose(1, 0, 2).reshape(NSH, K).astype(np.float32))
    return np.concatenate(outs, axis=0)


def kernel(inputs, clusters):
    from concourse.bass_utils import run_bass_kernel_spmd

    nc = _get_nc()
    in_maps = prep_in_maps(inputs, clusters)
    res = run_bass_kernel_spmd(nc, in_maps, core_ids=list(range(NCORES)))
    return unshard(res.results)
